# revision 16
# baseline (speedup 1.0000x reference)
"""Trainium2 Bass kernel for nn_AttentionModel (B=4, C=128, H=W=64).

Self-attention over spatial positions with 1x1-conv QKV projections and a
gamma-scaled residual:
    out = gamma * softmax(Q K / sqrt(C)) V + x

Sharding: data-parallel over batch (4 samples) x sequence-parallel over
query rows (2 halves of N=4096) = 8 NeuronCores. Each core holds the full
[C,C] weights, computes K/V for its whole sample, and the attention output
for its 2048 query rows.

Per-core algorithm (all matmuls bf16 with fp32 PSUM accumulate):
  QT[c,n] = WqT.T @ xf (+bq)   (pre-scaled by 1/sqrt(C) on host)
  K [c,m] = WkT.T @ xf (+bk)
  V [m,c] = xf_chunk.T @ WvT    (32 chunks of 128 rows; bv folded at end)
  per 512-wide group of query rows n, per 128-chunk of key index m:
    S^T[m,n] = K_chunk.T @ QT_group        (PE)
    P^T      = exp(S^T)                    (ACT, bf16 out)
    rowsum  += ones.T @ P^T                (PE, PSUM accumulate, bcast to 128p)
    pvacc   += V_chunk.T @ P^T             (PE, PSUM accumulate)
  recip = exp(-ln(rowsum))                 (ACT; Ln+Exp share a table set)
  out = (pvacc * recip + bv) * gamma + x   (DVE)

The softmax skips max-subtraction: energies are ~N(0,1) here, exp is safe.
"""

import numpy as np
import ml_dtypes

import concourse.bass as bass
import concourse.mybir as mybir
import concourse.tile as tile
from concourse import bacc
from concourse.bass_utils import run_bass_kernel_spmd

B, C, H, W = 4, 128, 64, 64
N = H * W            # 4096 spatial positions
NCORES = 8
RQ = N * B // NCORES  # 2048 query rows per core
NG = 512             # query-row group width (PSUM bank)
MC = 128             # key-chunk width (PE contraction)
F32 = mybir.dt.float32
BF16 = mybir.dt.bfloat16
AF = mybir.ActivationFunctionType


def build_bass():
    nc = bacc.Bacc("TRN2", target_bir_lowering=False, debug=False,
                   num_devices=NCORES)

    xf = nc.dram_tensor("xf", [C, N], BF16, kind="ExternalInput")
    xq = nc.dram_tensor("xq", [C, RQ], BF16, kind="ExternalInput")
    xr = nc.dram_tensor("xr", [C, RQ], F32, kind="ExternalInput")
    wct = nc.dram_tensor("wct", [C, 3, C], BF16, kind="ExternalInput")
    bb = nc.dram_tensor("bb", [C, 4], F32, kind="ExternalInput")
    out = nc.dram_tensor("out", [C, RQ], F32, kind="ExternalOutput")

    n_mc = N // MC       # 32 key chunks
    NSG = 1024           # query supergroup width
    n_sg = RQ // NSG     # 2 supergroups

    with tile.TileContext(nc) as tc:
        with tc.tile_pool(name="const", bufs=1) as cp:
            xf_t = cp.tile([C, N], BF16, tag="xf")
            xq_t = cp.tile([C, RQ], BF16, tag="xq")
            xr_t = cp.tile([C, RQ], F32, tag="xr")
            wc_t = cp.tile([C, 3, C], BF16, tag="wc")
            bb_t = cp.tile([C, 4], F32, tag="bb")
            ones_t = cp.tile([C, C], BF16, tag="ones")
            kk_t = cp.tile([C, N], BF16, tag="kk")
            qt_t = cp.tile([C, RQ], BF16, tag="qt")
            vv_t = cp.tile([C, n_mc, MC], BF16, tag="vv")
            wq_t, wk_t, wv_t = wc_t[:, 0, :], wc_t[:, 1, :], wc_t[:, 2, :]
            bq_t, bk_t = bb_t[:, 0:1], bb_t[:, 1:2]
            bvg_t, gm_t = bb_t[:, 2:3], bb_t[:, 3:4]

            # Small/urgent loads on the HWDGE (sync) queue; bulk x loads on
            # the SWDGE (gpsimd) queue so the two issue streams overlap.
            warm = cp.tile([C, 1], F32, tag="warm")
            nc.gpsimd.memset(warm[:], 0.0)
            nc.scalar.activation(warm[:], warm[:], AF.Exp)
            nc.sync.dma_start(wc_t[:], wct[:])
            nc.sync.dma_start(bb_t[:], bb[:])
            nc.sync.dma_start(xq_t[:], xq[:])
            nc.vector.memset(ones_t[:], 1.0)
            # first 512 columns separately: they gate the K0 projection
            nc.scalar.dma_start(xf_t[:, bass.ds(0, NG)], xf[:, bass.ds(0, NG)])
            nc.scalar.dma_start(xf_t[:, bass.ds(NG, N - NG)],
                                xf[:, bass.ds(NG, N - NG)])

            with (
                tc.tile_pool(name="stp", bufs=2,
                             space=bass.MemorySpace.PSUM) as stp,
                tc.tile_pool(name="pvp", bufs=1,
                             space=bass.MemorySpace.PSUM) as pvp,
                tc.tile_pool(name="vpp", bufs=2,
                             space=bass.MemorySpace.PSUM) as vpp,
                tc.tile_pool(name="ptp", bufs=10) as ptp,
                tc.tile_pool(name="accp", bufs=2) as accp,
                tc.tile_pool(name="fin", bufs=2) as fin,
            ):
                def proj(dst, w, src, bias, on_act=False):
                    ps = vpp.tile([C, NG], F32, tag="vp")
                    nc.tensor.matmul(ps[:], w, src, start=True, stop=True)
                    if on_act:
                        nc.scalar.activation(dst, ps[:], AF.Identity,
                                             bias=bias)
                    else:
                        nc.vector.tensor_scalar_add(out=dst, in0=ps[:],
                                                    scalar1=bias)

                def vbatch(mc0):
                    # V projection for key chunks mc0..mc0+3 in one PSUM
                    # tile, one PSUM->SBUF copy
                    vp = vpp.tile([C, NG], F32, tag="vp")
                    for i in range(4):
                        nc.tensor.matmul(vp[:, bass.ts(i, MC)],
                                         xf_t[:, bass.ts(mc0 + i, MC)],
                                         wv_t, start=True, stop=True)
                    nc.vector.tensor_copy(vv_t[:, mc0:mc0 + 4, :], vp[:])

                # only what the first S^T matmul needs (ACT is idle here);
                # the rest is emitted just-in-time inside the sg0 loop
                proj(qt_t[:, bass.ts(0, NG)], wq_t, xq_t[:, bass.ts(0, NG)],
                     bq_t, on_act=True)
                proj(qt_t[:, bass.ts(1, NG)], wq_t, xq_t[:, bass.ts(1, NG)],
                     bq_t, on_act=True)
                proj(kk_t[:, bass.ts(0, NG)], wk_t, xf_t[:, bass.ts(0, NG)],
                     bk_t, on_act=True)

                for sg in range(n_sg):
                    pv_ps = pvp.tile([C, NSG], F32, tag="pv")
                    acc_d = accp.tile([C, NSG], BF16, tag="acc_d")
                    acc_g = accp.tile([C, NSG], BF16, tag="acc_g")
                    for mc in range(n_mc):
                        st_ps = stp.tile([C, NSG], F32, tag="st")
                        for q in range(NSG // NG):
                            nn = sg * NSG + q * NG
                            nc.tensor.matmul(
                                st_ps[:, bass.ts(q, NG)],
                                kk_t[:, bass.ts(mc, MC)],
                                qt_t[:, bass.ds(nn, NG)],
                                start=True, stop=True)
                        pt = ptp.tile([C, NSG], BF16, tag="pt")
                        nc.scalar.activation(pt[:], st_ps[:], AF.Exp)
                        if sg == 0:
                            if mc == 0:
                                vbatch(0)
                            # just-in-time work for upcoming chunks, placed
                            # on units whose row-sum add runs on GPSIMD so
                            # the DVE load stays level
                            if mc % 4 == 1 and (mc + 3) // 4 < N // NG:
                                j = (mc + 3) // 4
                                js = bass.ts(j, NG)
                                proj(kk_t[:, js], wk_t, xf_t[:, js], bk_t)
                            if mc in (23, 27):
                                j = 2 + (mc - 23) // 4
                                proj(qt_t[:, bass.ts(j, NG)], wq_t,
                                     xq_t[:, bass.ts(j, NG)], bq_t)
                            if mc % 4 == 0 and mc + 4 < n_mc:
                                vbatch(mc + 4)
                            if mc == 4:
                                nc.gpsimd.dma_start(xr_t[:], xr[:])
                        for q in range(NSG // NG):
                            nc.tensor.matmul(
                                pv_ps[:, bass.ts(q, NG)],
                                vv_t[:, mc, :], pt[:, bass.ts(q, NG)],
                                start=(mc == 0), stop=(mc == n_mc - 1))
                        acc = acc_g if mc % 2 == 1 else acc_d
                        if mc < 2:
                            nc.vector.tensor_copy(acc[:], pt[:])
                        else:
                            nc.vector.tensor_add(acc[:], acc[:], pt[:])

                    # rowsum = ones.T @ acc_g + ones.T @ acc_d (acc_g is
                    # final after chunk 27 so its matmuls overlap the tail
                    # chunks; acc_d finishes right after the last exp)
                    rb = fin.tile([C, NSG], F32, tag="rb")
                    t1 = fin.tile([C, NSG], F32, tag="t1")
                    o3 = fin.tile([C, NSG], F32, tag="o3")
                    for q in range(NSG // NG):
                        s = bass.ts(q, NG)
                        nn = bass.ds(sg * NSG + q * NG, NG)
                        rs_ps = vpp.tile([C, NG], F32, tag="vp")
                        nc.tensor.matmul(rs_ps[:], ones_t[:],
                                         acc_g[:, s], start=True, stop=False)
                        nc.tensor.matmul(rs_ps[:], ones_t[:],
                                         acc_d[:, s], start=False, stop=True)
                        nc.vector.reciprocal_approx_fast(out=rb[:, s],
                                                         in_=rs_ps[:])
                        nc.vector.scalar_tensor_tensor(
                            out=t1[:, s], in0=pv_ps[:, s], scalar=gm_t,
                            in1=rb[:, s],
                            op0=mybir.AluOpType.mult, op1=mybir.AluOpType.mult)
                        nc.vector.scalar_tensor_tensor(
                            out=o3[:, s], in0=t1[:, s], scalar=bvg_t,
                            in1=xr_t[:, nn],
                            op0=mybir.AluOpType.add, op1=mybir.AluOpType.add)
                        nc.sync.dma_start(out[:, nn], o3[:, s])

    nc.compile()
    return nc


_NC_CACHE = None


def _get_nc():
    global _NC_CACHE
    if _NC_CACHE is None:
        _NC_CACHE = build_bass()
    return _NC_CACHE


def make_in_maps(x, Wq, bq, Wk, bk, Wv, bv, gamma):
    x = np.asarray(x, dtype=np.float32)
    Wq = np.asarray(Wq, dtype=np.float32)
    Wk = np.asarray(Wk, dtype=np.float32)
    Wv = np.asarray(Wv, dtype=np.float32)
    bq = np.asarray(bq, dtype=np.float32)
    bk = np.asarray(bk, dtype=np.float32)
    bv = np.asarray(bv, dtype=np.float32)
    gamma = np.asarray(gamma, dtype=np.float32)

    scale = np.float32(1.0 / np.sqrt(C))
    xf = x.reshape(B, C, N)
    wct_s = np.ascontiguousarray(
        np.stack([(Wq * scale).T, Wk.T, Wv.T], axis=1)
    ).astype(ml_dtypes.bfloat16)  # [C_in, 3, C_out]
    g0 = np.float32(gamma.reshape(-1)[0])
    bb_s = np.ascontiguousarray(
        np.stack([bq * scale, bk, bv * g0, np.full(C, g0, np.float32)],
                 axis=1)).astype(np.float32)

    in_maps = []
    for core in range(NCORES):
        b, h = core // 2, core % 2
        xslice = np.ascontiguousarray(xf[b][:, h * RQ:(h + 1) * RQ])
        in_maps.append({
            "xf": xf[b].astype(ml_dtypes.bfloat16),
            "xq": xslice.astype(ml_dtypes.bfloat16),
            "xr": xslice,
            "wct": wct_s, "bb": bb_s,
        })
    return in_maps


def assemble(results):
    out = np.empty((B, C, N), dtype=np.float32)
    for core in range(NCORES):
        b, h = core // 2, core % 2
        out[b][:, h * RQ:(h + 1) * RQ] = results[core]["out"]
    return out.reshape(B, C, H, W)


def run(inputs: dict, trace: bool = False, tmpdir: str | None = None):
    nc = _get_nc()
    in_maps = make_in_maps(**inputs)
    res = run_bass_kernel_spmd(nc, in_maps, core_ids=list(range(NCORES)),
                               trace=trace, tmpdir=tmpdir)
    return assemble(res.results), res


def kernel(**inputs) -> np.ndarray:
    out, _ = run(inputs, trace=False)
    return out


# revision 17
# speedup vs baseline: 1.0193x; 1.0193x over previous
"""Trainium2 Bass kernel for nn_AttentionModel (B=4, C=128, H=W=64).

Self-attention over spatial positions with 1x1-conv QKV projections and a
gamma-scaled residual:
    out = gamma * softmax(Q K / sqrt(C)) V + x

Sharding: data-parallel over batch (4 samples) x sequence-parallel over
query rows (2 halves of N=4096) = 8 NeuronCores. Each core holds the full
[C,C] weights, computes K/V for its whole sample, and the attention output
for its 2048 query rows.

Per-core algorithm (all matmuls bf16 with fp32 PSUM accumulate):
  QT[c,n] = WqT.T @ xf (+bq)   (pre-scaled by 1/sqrt(C) on host)
  K [c,m] = WkT.T @ xf (+bk)
  V [m,c] = xf_chunk.T @ WvT    (32 chunks of 128 rows; bv folded at end)
  per 512-wide group of query rows n, per 128-chunk of key index m:
    S^T[m,n] = K_chunk.T @ QT_group        (PE)
    P^T      = exp(S^T)                    (ACT, bf16 out)
    rowsum  += ones.T @ P^T                (PE, PSUM accumulate, bcast to 128p)
    pvacc   += V_chunk.T @ P^T             (PE, PSUM accumulate)
  recip = exp(-ln(rowsum))                 (ACT; Ln+Exp share a table set)
  out = (pvacc * recip + bv) * gamma + x   (DVE)

The softmax skips max-subtraction: energies are ~N(0,1) here, exp is safe.
"""

import numpy as np
import ml_dtypes

import concourse.bass as bass
import concourse.mybir as mybir
import concourse.tile as tile
from concourse import bacc
from concourse.bass_utils import run_bass_kernel_spmd

B, C, H, W = 4, 128, 64, 64
N = H * W            # 4096 spatial positions
NCORES = 8
RQ = N * B // NCORES  # 2048 query rows per core
NG = 512             # query-row group width (PSUM bank)
MC = 128             # key-chunk width (PE contraction)
F32 = mybir.dt.float32
BF16 = mybir.dt.bfloat16
AF = mybir.ActivationFunctionType


def build_bass():
    nc = bacc.Bacc("TRN2", target_bir_lowering=False, debug=False,
                   num_devices=NCORES)

    xf = nc.dram_tensor("xf", [C, N], BF16, kind="ExternalInput")
    xr = nc.dram_tensor("xr", [C, RQ], F32, kind="ExternalInput")
    wct = nc.dram_tensor("wct", [C, 3, C], BF16, kind="ExternalInput")
    bb = nc.dram_tensor("bb", [C, 4], F32, kind="ExternalInput")
    out = nc.dram_tensor("out", [C, RQ], F32, kind="ExternalOutput")

    n_mc = N // MC       # 32 key chunks
    NSG = 1024           # query supergroup width
    n_sg = RQ // NSG     # 2 supergroups

    with tile.TileContext(nc) as tc:
        with tc.tile_pool(name="const", bufs=1) as cp:
            xf_t = cp.tile([C, N], BF16, tag="xf")
            xr_t = cp.tile([C, RQ], F32, tag="xr")
            wc_t = cp.tile([C, 3, C], BF16, tag="wc")
            bb_t = cp.tile([C, 4], F32, tag="bb")
            ones_t = cp.tile([C, C], BF16, tag="ones")
            kk_t = cp.tile([C, N], BF16, tag="kk")
            qt_t = cp.tile([C, RQ], BF16, tag="qt")
            vv_t = cp.tile([C, n_mc, MC], BF16, tag="vv")
            wq_t, wk_t, wv_t = wc_t[:, 0, :], wc_t[:, 1, :], wc_t[:, 2, :]
            bq_t, bk_t = bb_t[:, 0:1], bb_t[:, 1:2]
            bvg_t, gm_t = bb_t[:, 2:3], bb_t[:, 3:4]

            # Small/urgent loads on the HWDGE (sync) queue; bulk x loads on
            # the SWDGE (gpsimd) queue so the two issue streams overlap.
            warm = cp.tile([C, 1], F32, tag="warm")
            nc.gpsimd.memset(warm[:], 0.0)
            nc.scalar.activation(warm[:], warm[:], AF.Exp)
            nc.sync.dma_start(wc_t[:], wct[:])
            nc.sync.dma_start(bb_t[:], bb[:])
            nc.vector.memset(ones_t[:], 1.0)
            # xf is pre-rotated per core so the query block is cols 0:2048;
            # chunked loads across both HWDGE queues release deps early
            NXF = N // 4
            nc.sync.dma_start(xf_t[:, bass.ts(0, NXF)], xf[:, bass.ts(0, NXF)])
            nc.scalar.dma_start(xf_t[:, bass.ts(1, NXF)],
                                xf[:, bass.ts(1, NXF)])
            nc.sync.dma_start(xf_t[:, bass.ts(2, NXF)], xf[:, bass.ts(2, NXF)])
            nc.scalar.dma_start(xf_t[:, bass.ts(3, NXF)],
                                xf[:, bass.ts(3, NXF)])

            with (
                tc.tile_pool(name="stp", bufs=2,
                             space=bass.MemorySpace.PSUM) as stp,
                tc.tile_pool(name="pvp", bufs=1,
                             space=bass.MemorySpace.PSUM) as pvp,
                tc.tile_pool(name="vpp", bufs=2,
                             space=bass.MemorySpace.PSUM) as vpp,
                tc.tile_pool(name="ptp", bufs=10) as ptp,
                tc.tile_pool(name="accp", bufs=2) as accp,
                tc.tile_pool(name="fin", bufs=2) as fin,
            ):
                def proj(dst, w, src, bias, on_act=False):
                    ps = vpp.tile([C, NG], F32, tag="vp")
                    nc.tensor.matmul(ps[:], w, src, start=True, stop=True)
                    if on_act:
                        nc.scalar.activation(dst, ps[:], AF.Identity,
                                             bias=bias)
                    else:
                        nc.vector.tensor_scalar_add(out=dst, in0=ps[:],
                                                    scalar1=bias)

                def vbatch(mc0):
                    # V projection for key chunks mc0..mc0+3 in one PSUM
                    # tile, one PSUM->SBUF copy
                    vp = vpp.tile([C, NG], F32, tag="vp")
                    for i in range(4):
                        nc.tensor.matmul(vp[:, bass.ts(i, MC)],
                                         xf_t[:, bass.ts(mc0 + i, MC)],
                                         wv_t, start=True, stop=True)
                    nc.vector.tensor_copy(vv_t[:, mc0:mc0 + 4, :], vp[:])

                # only what the first S^T matmul needs (ACT is idle here);
                # the rest is emitted just-in-time inside the sg0 loop
                proj(qt_t[:, bass.ts(0, NG)], wq_t, xf_t[:, bass.ts(0, NG)],
                     bq_t, on_act=True)
                proj(qt_t[:, bass.ts(1, NG)], wq_t, xf_t[:, bass.ts(1, NG)],
                     bq_t, on_act=True)
                proj(kk_t[:, bass.ts(0, NG)], wk_t, xf_t[:, bass.ts(0, NG)],
                     bk_t, on_act=True)

                for sg in range(n_sg):
                    pv_ps = pvp.tile([C, NSG], F32, tag="pv")
                    acc_d = accp.tile([C, NSG], BF16, tag="acc_d")
                    acc_g = accp.tile([C, NSG], BF16, tag="acc_g")
                    for mc in range(n_mc):
                        st_ps = stp.tile([C, NSG], F32, tag="st")
                        for q in range(NSG // NG):
                            nn = sg * NSG + q * NG
                            nc.tensor.matmul(
                                st_ps[:, bass.ts(q, NG)],
                                kk_t[:, bass.ts(mc, MC)],
                                qt_t[:, bass.ds(nn, NG)],
                                start=True, stop=True)
                        pt = ptp.tile([C, NSG], BF16, tag="pt")
                        nc.scalar.activation(pt[:], st_ps[:], AF.Exp)
                        if sg == 0:
                            if mc == 0:
                                vbatch(0)
                            # just-in-time work for upcoming chunks, placed
                            # on units whose row-sum add runs on GPSIMD so
                            # the DVE load stays level
                            if mc % 4 == 1 and (mc + 3) // 4 < N // NG:
                                j = (mc + 3) // 4
                                js = bass.ts(j, NG)
                                proj(kk_t[:, js], wk_t, xf_t[:, js], bk_t)
                            if mc in (23, 27):
                                j = 2 + (mc - 23) // 4
                                proj(qt_t[:, bass.ts(j, NG)], wq_t,
                                     xf_t[:, bass.ts(j, NG)], bq_t)
                            if mc % 4 == 0 and mc + 4 < n_mc:
                                vbatch(mc + 4)
                            if mc == 4:
                                nc.gpsimd.dma_start(xr_t[:], xr[:])
                        for q in range(NSG // NG):
                            nc.tensor.matmul(
                                pv_ps[:, bass.ts(q, NG)],
                                vv_t[:, mc, :], pt[:, bass.ts(q, NG)],
                                start=(mc == 0), stop=(mc == n_mc - 1))
                        acc = acc_g if mc % 2 == 1 else acc_d
                        if mc < 2:
                            nc.vector.tensor_copy(acc[:], pt[:])
                        else:
                            nc.vector.tensor_add(acc[:], acc[:], pt[:])

                    # rowsum = ones.T @ acc_g + ones.T @ acc_d (acc_g is
                    # final after chunk 27 so its matmuls overlap the tail
                    # chunks; acc_d finishes right after the last exp)
                    rb = fin.tile([C, NSG], F32, tag="rb")
                    t1 = fin.tile([C, NSG], F32, tag="t1")
                    o3 = fin.tile([C, NSG], F32, tag="o3")
                    for q in range(NSG // NG):
                        s = bass.ts(q, NG)
                        nn = bass.ds(sg * NSG + q * NG, NG)
                        rs_ps = vpp.tile([C, NG], F32, tag="vp")
                        nc.tensor.matmul(rs_ps[:], ones_t[:],
                                         acc_g[:, s], start=True, stop=False)
                        nc.tensor.matmul(rs_ps[:], ones_t[:],
                                         acc_d[:, s], start=False, stop=True)
                        nc.vector.reciprocal_approx_fast(out=rb[:, s],
                                                         in_=rs_ps[:])
                        nc.vector.scalar_tensor_tensor(
                            out=t1[:, s], in0=pv_ps[:, s], scalar=gm_t,
                            in1=rb[:, s],
                            op0=mybir.AluOpType.mult, op1=mybir.AluOpType.mult)
                        nc.vector.scalar_tensor_tensor(
                            out=o3[:, s], in0=t1[:, s], scalar=bvg_t,
                            in1=xr_t[:, nn],
                            op0=mybir.AluOpType.add, op1=mybir.AluOpType.add)
                        nc.sync.dma_start(out[:, nn], o3[:, s])

    nc.compile()
    return nc


_NC_CACHE = None


def _get_nc():
    global _NC_CACHE
    if _NC_CACHE is None:
        _NC_CACHE = build_bass()
    return _NC_CACHE


def make_in_maps(x, Wq, bq, Wk, bk, Wv, bv, gamma):
    x = np.asarray(x, dtype=np.float32)
    Wq = np.asarray(Wq, dtype=np.float32)
    Wk = np.asarray(Wk, dtype=np.float32)
    Wv = np.asarray(Wv, dtype=np.float32)
    bq = np.asarray(bq, dtype=np.float32)
    bk = np.asarray(bk, dtype=np.float32)
    bv = np.asarray(bv, dtype=np.float32)
    gamma = np.asarray(gamma, dtype=np.float32)

    scale = np.float32(1.0 / np.sqrt(C))
    xf = x.reshape(B, C, N)
    wct_s = np.ascontiguousarray(
        np.stack([(Wq * scale).T, Wk.T, Wv.T], axis=1)
    ).astype(ml_dtypes.bfloat16)  # [C_in, 3, C_out]
    g0 = np.float32(gamma.reshape(-1)[0])
    bb_s = np.ascontiguousarray(
        np.stack([bq * scale, bk, bv * g0, np.full(C, g0, np.float32)],
                 axis=1)).astype(np.float32)

    in_maps = []
    for core in range(NCORES):
        b, h = core // 2, core % 2
        xrot = np.roll(xf[b], -h * RQ, axis=1)
        in_maps.append({
            "xf": np.ascontiguousarray(xrot).astype(ml_dtypes.bfloat16),
            "xr": np.ascontiguousarray(xrot[:, :RQ]),
            "wct": wct_s, "bb": bb_s,
        })
    return in_maps


def assemble(results):
    out = np.empty((B, C, N), dtype=np.float32)
    for core in range(NCORES):
        b, h = core // 2, core % 2
        out[b][:, h * RQ:(h + 1) * RQ] = results[core]["out"]
    return out.reshape(B, C, H, W)


def run(inputs: dict, trace: bool = False, tmpdir: str | None = None):
    nc = _get_nc()
    in_maps = make_in_maps(**inputs)
    res = run_bass_kernel_spmd(nc, in_maps, core_ids=list(range(NCORES)),
                               trace=trace, tmpdir=tmpdir)
    return assemble(res.results), res


def kernel(**inputs) -> np.ndarray:
    out, _ = run(inputs, trace=False)
    return out


# revision 18
# speedup vs baseline: 1.0333x; 1.0137x over previous
"""Trainium2 Bass kernel for nn_AttentionModel (B=4, C=128, H=W=64).

Self-attention over spatial positions with 1x1-conv QKV projections and a
gamma-scaled residual:
    out = gamma * softmax(Q K / sqrt(C)) V + x

Sharding: data-parallel over batch (4 samples) x sequence-parallel over
query rows (2 halves of N=4096) = 8 NeuronCores. Each core holds the full
[C,C] weights, computes K/V for its whole sample, and the attention output
for its 2048 query rows.

Per-core algorithm (all matmuls bf16 with fp32 PSUM accumulate):
  QT[c,n] = WqT.T @ xf (+bq)   (pre-scaled by 1/sqrt(C) on host)
  K [c,m] = WkT.T @ xf (+bk)
  V [m,c] = xf_chunk.T @ WvT    (32 chunks of 128 rows; bv folded at end)
  per 512-wide group of query rows n, per 128-chunk of key index m:
    S^T[m,n] = K_chunk.T @ QT_group        (PE)
    P^T      = exp(S^T)                    (ACT, bf16 out)
    rowsum  += ones.T @ P^T                (PE, PSUM accumulate, bcast to 128p)
    pvacc   += V_chunk.T @ P^T             (PE, PSUM accumulate)
  recip = exp(-ln(rowsum))                 (ACT; Ln+Exp share a table set)
  out = (pvacc * recip + bv) * gamma + x   (DVE)

The softmax skips max-subtraction: energies are ~N(0,1) here, exp is safe.
"""

import numpy as np
import ml_dtypes

import concourse.bass as bass
import concourse.mybir as mybir
import concourse.tile as tile
from concourse import bacc
from concourse.bass_utils import run_bass_kernel_spmd

B, C, H, W = 4, 128, 64, 64
N = H * W            # 4096 spatial positions
NCORES = 8
RQ = N * B // NCORES  # 2048 query rows per core
NG = 512             # query-row group width (PSUM bank)
MC = 128             # key-chunk width (PE contraction)
F32 = mybir.dt.float32
BF16 = mybir.dt.bfloat16
AF = mybir.ActivationFunctionType


def build_bass():
    nc = bacc.Bacc("TRN2", target_bir_lowering=False, debug=False,
                   num_devices=NCORES)

    xf = nc.dram_tensor("xf", [C, N], BF16, kind="ExternalInput")
    xr = nc.dram_tensor("xr", [C, RQ], F32, kind="ExternalInput")
    wct = nc.dram_tensor("wct", [C, 3, C], BF16, kind="ExternalInput")
    bb = nc.dram_tensor("bb", [C, 4], F32, kind="ExternalInput")
    out = nc.dram_tensor("out", [C, RQ], F32, kind="ExternalOutput")

    n_mc = N // MC       # 32 key chunks
    NSG = 1024           # query supergroup width
    n_sg = RQ // NSG     # 2 supergroups

    with tile.TileContext(nc) as tc:
        with tc.tile_pool(name="const", bufs=1) as cp:
            xf_t = cp.tile([C, N], BF16, tag="xf")
            xr_t = cp.tile([C, RQ], F32, tag="xr")
            wc_t = cp.tile([C, 3, C], BF16, tag="wc")
            bb_t = cp.tile([C, 4], F32, tag="bb")
            ones_t = cp.tile([C, C], BF16, tag="ones")
            kk_t = cp.tile([C, N], BF16, tag="kk")
            qt_t = cp.tile([C, RQ], BF16, tag="qt")
            vv_t = cp.tile([C, n_mc, MC], BF16, tag="vv")
            wq_t, wk_t, wv_t = wc_t[:, 0, :], wc_t[:, 1, :], wc_t[:, 2, :]
            bq_t, bk_t = bb_t[:, 0:1], bb_t[:, 1:2]
            bvg_t, gm_t = bb_t[:, 2:3], bb_t[:, 3:4]

            # Small/urgent loads on the HWDGE (sync) queue; bulk x loads on
            # the SWDGE (gpsimd) queue so the two issue streams overlap.
            warm = cp.tile([C, 1], F32, tag="warm")
            nc.gpsimd.memset(warm[:], 0.0)
            nc.scalar.activation(warm[:], warm[:], AF.Exp)
            nc.sync.dma_start(wc_t[:], wct[:])
            nc.sync.dma_start(bb_t[:], bb[:])
            nc.vector.memset(ones_t[:], 1.0)
            # xf is pre-rotated per core so the query block is cols 0:2048;
            # small chunks alternating across both HWDGE queues release
            # dependencies early (the first chunk gates the whole pipeline)
            for j in range(8):
                eng = nc.sync if j % 2 == 0 else nc.scalar
                eng.dma_start(xf_t[:, bass.ts(j, NG)], xf[:, bass.ts(j, NG)])

            with (
                tc.tile_pool(name="stp", bufs=2,
                             space=bass.MemorySpace.PSUM) as stp,
                tc.tile_pool(name="pvp", bufs=1,
                             space=bass.MemorySpace.PSUM) as pvp,
                tc.tile_pool(name="vpp", bufs=2,
                             space=bass.MemorySpace.PSUM) as vpp,
                tc.tile_pool(name="ptp", bufs=10) as ptp,
                tc.tile_pool(name="accp", bufs=2) as accp,
                tc.tile_pool(name="fin", bufs=2) as fin,
            ):
                def proj(dst, w, src, bias, on_act=False):
                    ps = vpp.tile([C, NG], F32, tag="vp")
                    nc.tensor.matmul(ps[:], w, src, start=True, stop=True)
                    if on_act:
                        nc.scalar.activation(dst, ps[:], AF.Identity,
                                             bias=bias)
                    else:
                        nc.vector.tensor_scalar_add(out=dst, in0=ps[:],
                                                    scalar1=bias)

                def vbatch(mc0):
                    # V projection for key chunks mc0..mc0+3 in one PSUM
                    # tile, one PSUM->SBUF copy
                    vp = vpp.tile([C, NG], F32, tag="vp")
                    for i in range(4):
                        nc.tensor.matmul(vp[:, bass.ts(i, MC)],
                                         xf_t[:, bass.ts(mc0 + i, MC)],
                                         wv_t, start=True, stop=True)
                    nc.vector.tensor_copy(vv_t[:, mc0:mc0 + 4, :], vp[:])

                # only what the first S^T matmul needs (ACT is idle here);
                # the rest is emitted just-in-time inside the sg0 loop
                proj(qt_t[:, bass.ts(0, NG)], wq_t, xf_t[:, bass.ts(0, NG)],
                     bq_t, on_act=True)
                proj(qt_t[:, bass.ts(1, NG)], wq_t, xf_t[:, bass.ts(1, NG)],
                     bq_t, on_act=True)
                proj(kk_t[:, bass.ts(0, NG)], wk_t, xf_t[:, bass.ts(0, NG)],
                     bk_t, on_act=True)

                for sg in range(n_sg):
                    pv_ps = pvp.tile([C, NSG], F32, tag="pv")
                    acc_d = accp.tile([C, NSG], BF16, tag="acc_d")
                    acc_g = accp.tile([C, NSG], BF16, tag="acc_g")
                    for mc in range(n_mc):
                        st_ps = stp.tile([C, NSG], F32, tag="st")
                        for q in range(NSG // NG):
                            nn = sg * NSG + q * NG
                            nc.tensor.matmul(
                                st_ps[:, bass.ts(q, NG)],
                                kk_t[:, bass.ts(mc, MC)],
                                qt_t[:, bass.ds(nn, NG)],
                                start=True, stop=True)
                        pt = ptp.tile([C, NSG], BF16, tag="pt")
                        nc.scalar.activation(pt[:], st_ps[:], AF.Exp)
                        if sg == 0:
                            if mc == 0:
                                vbatch(0)
                            # just-in-time work for upcoming chunks, placed
                            # on units whose row-sum add runs on GPSIMD so
                            # the DVE load stays level
                            if mc % 4 == 1 and (mc + 3) // 4 < N // NG:
                                j = (mc + 3) // 4
                                js = bass.ts(j, NG)
                                proj(kk_t[:, js], wk_t, xf_t[:, js], bk_t)
                            if mc in (23, 27):
                                j = 2 + (mc - 23) // 4
                                proj(qt_t[:, bass.ts(j, NG)], wq_t,
                                     xf_t[:, bass.ts(j, NG)], bq_t)
                            if mc % 4 == 0 and mc + 4 < n_mc:
                                vbatch(mc + 4)
                            if mc == 4:
                                nc.gpsimd.dma_start(xr_t[:], xr[:])
                        for q in range(NSG // NG):
                            nc.tensor.matmul(
                                pv_ps[:, bass.ts(q, NG)],
                                vv_t[:, mc, :], pt[:, bass.ts(q, NG)],
                                start=(mc == 0), stop=(mc == n_mc - 1))
                        acc = acc_g if mc % 2 == 1 else acc_d
                        if mc < 2:
                            nc.vector.tensor_copy(acc[:], pt[:])
                        else:
                            nc.vector.tensor_add(acc[:], acc[:], pt[:])

                    # rowsum = ones.T @ acc_g + ones.T @ acc_d (acc_g is
                    # final after chunk 27 so its matmuls overlap the tail
                    # chunks; acc_d finishes right after the last exp)
                    rb = fin.tile([C, NSG], F32, tag="rb")
                    t1 = fin.tile([C, NSG], F32, tag="t1")
                    o3 = fin.tile([C, NSG], F32, tag="o3")
                    for q in range(NSG // NG):
                        s = bass.ts(q, NG)
                        nn = bass.ds(sg * NSG + q * NG, NG)
                        rs_ps = vpp.tile([C, NG], F32, tag="vp")
                        nc.tensor.matmul(rs_ps[:], ones_t[:],
                                         acc_g[:, s], start=True, stop=False)
                        nc.tensor.matmul(rs_ps[:], ones_t[:],
                                         acc_d[:, s], start=False, stop=True)
                        nc.vector.reciprocal_approx_fast(out=rb[:, s],
                                                         in_=rs_ps[:])
                        nc.vector.scalar_tensor_tensor(
                            out=t1[:, s], in0=pv_ps[:, s], scalar=gm_t,
                            in1=rb[:, s],
                            op0=mybir.AluOpType.mult, op1=mybir.AluOpType.mult)
                        nc.vector.scalar_tensor_tensor(
                            out=o3[:, s], in0=t1[:, s], scalar=bvg_t,
                            in1=xr_t[:, nn],
                            op0=mybir.AluOpType.add, op1=mybir.AluOpType.add)
                        nc.sync.dma_start(out[:, nn], o3[:, s])

    nc.compile()
    return nc


_NC_CACHE = None


def _get_nc():
    global _NC_CACHE
    if _NC_CACHE is None:
        _NC_CACHE = build_bass()
    return _NC_CACHE


def make_in_maps(x, Wq, bq, Wk, bk, Wv, bv, gamma):
    x = np.asarray(x, dtype=np.float32)
    Wq = np.asarray(Wq, dtype=np.float32)
    Wk = np.asarray(Wk, dtype=np.float32)
    Wv = np.asarray(Wv, dtype=np.float32)
    bq = np.asarray(bq, dtype=np.float32)
    bk = np.asarray(bk, dtype=np.float32)
    bv = np.asarray(bv, dtype=np.float32)
    gamma = np.asarray(gamma, dtype=np.float32)

    scale = np.float32(1.0 / np.sqrt(C))
    xf = x.reshape(B, C, N)
    wct_s = np.ascontiguousarray(
        np.stack([(Wq * scale).T, Wk.T, Wv.T], axis=1)
    ).astype(ml_dtypes.bfloat16)  # [C_in, 3, C_out]
    g0 = np.float32(gamma.reshape(-1)[0])
    bb_s = np.ascontiguousarray(
        np.stack([bq * scale, bk, bv * g0, np.full(C, g0, np.float32)],
                 axis=1)).astype(np.float32)

    in_maps = []
    for core in range(NCORES):
        b, h = core // 2, core % 2
        xrot = np.roll(xf[b], -h * RQ, axis=1)
        in_maps.append({
            "xf": np.ascontiguousarray(xrot).astype(ml_dtypes.bfloat16),
            "xr": np.ascontiguousarray(xrot[:, :RQ]),
            "wct": wct_s, "bb": bb_s,
        })
    return in_maps


def assemble(results):
    out = np.empty((B, C, N), dtype=np.float32)
    for core in range(NCORES):
        b, h = core // 2, core % 2
        out[b][:, h * RQ:(h + 1) * RQ] = results[core]["out"]
    return out.reshape(B, C, H, W)


def run(inputs: dict, trace: bool = False, tmpdir: str | None = None):
    nc = _get_nc()
    in_maps = make_in_maps(**inputs)
    res = run_bass_kernel_spmd(nc, in_maps, core_ids=list(range(NCORES)),
                               trace=trace, tmpdir=tmpdir)
    return assemble(res.results), res


def kernel(**inputs) -> np.ndarray:
    out, _ = run(inputs, trace=False)
    return out
